# revision 1
# baseline (speedup 1.0000x reference)
"""Trainium2 Bass kernel for nn_NCFG_21139829031662 (gnn_message_passing).

RippleNet-style model: hop-0 seed-set sum + 2 hops of (gather triples,
attention softmax over K, 2-step tanh RNN, weighted sum), then a
user/item dot + sigmoid.

Strategy: pure data-parallel over the 4096-user batch across 8 cores
(512 users/core); embedding tables replicated in each core's HBM.
The dominant cost is SWDGE descriptor generation for the 128B random
entity gathers (~10.8 ns/row on the GpSimd Q7), so the kernel
eliminates every non-entity SWDGE instruction:
  - relation embeddings never touch SWDGE: r.r enters the logits as a
    host-precomputed per-token scalar (an index transform), and the RNN's
    Wr@r term is computed on the PE as WRtab^T @ onehot(r) accumulated
    straight into the RNN PSUM groups (onehot uploaded as bf16 0/1).
  - entity/hop0/final gathers use per-column indirect1d DMAs (the HW
    contract: 128 descriptors per instruction).
  - the final item embedding entity_emb[items] + rec_item_emb[items] is
    gathered once from a host-precombined table (same-index fold).

Per-core on-chip layout ("G-layout"): token (u, k) -> partition
p = (u%2)*64 + k, free column j = u//2 (32 f32 per column).
"""

import sys
import numpy as np

sys.path.insert(0, "/opt/trn_rl_repo")

# ---------------------------------------------------------------- constants
DIM = 32
N_ENTITY = 500000
N_RELATION = 64
N_USER = 100000
N_ITEM = 200000
B = 4096
K = 64
L = 2
NCORES = 8
P = 128


def build_core_program(BC=512, JB=32):
    """Build the single-core bass program (SPMD: same program on all cores).

    BC: users per core. JB: j-columns (user pairs) per processing batch.
    """
    import concourse.bass as bass
    import concourse.bacc as bacc
    import concourse.mybir as mybir
    import concourse.tile as tile
    from concourse.masks import make_identity

    J = BC // 2              # j-columns total
    NBATCH = J // JB         # batches per hop
    NCHUNK = J // 16         # 16-j output chunks
    NR = 2 * NCHUNK          # output psum rows
    assert J % JB == 0 and JB % 16 == 0
    CPB = JB // 16           # chunks per batch
    STB = JB // 4            # supertiles ([128,128] blocks) per batch
    f32 = mybir.dt.float32
    bf16 = mybir.dt.bfloat16
    i32 = mybir.dt.int32

    nc = bacc.Bacc("TRN2", target_bir_lowering=False, debug=False)

    # DRAM inputs
    entity = nc.dram_tensor("entity", [N_ENTITY, DIM], f32, kind="ExternalInput").ap()
    rec_user = nc.dram_tensor("rec_user", [N_USER, DIM], f32, kind="ExternalInput").ap()
    # host-precomputed entity_emb[:N_ITEM] + rec_item_emb (same-index fold)
    item_comb = nc.dram_tensor("item_comb", [N_ITEM, DIM], f32,
                               kind="ExternalInput").ap()
    idx_hop0 = nc.dram_tensor("idx_hop0", [P, J], i32, kind="ExternalInput").ap()
    idx_h = nc.dram_tensor("idx_h", [L, P, J], i32, kind="ExternalInput").ap()
    idx_t = nc.dram_tensor("idx_t", [L, P, J], i32, kind="ExternalInput").ap()
    rrsq_in = nc.dram_tensor("rrsq", [L, P, J], f32, kind="ExternalInput").ap()
    onehot_in = nc.dram_tensor("onehot", [L, N_RELATION, J * P], bf16,
                               kind="ExternalInput").ap()
    fin_users = nc.dram_tensor("fin_users", [P, 4], i32, kind="ExternalInput").ap()
    fin_items = nc.dram_tensor("fin_items", [P, 4], i32, kind="ExternalInput").ap()
    wh_bd = nc.dram_tensor("wh_bd", [P, P], f32, kind="ExternalInput").ap()
    whh_bd = nc.dram_tensor("whh_bd", [P, P], f32, kind="ExternalInput").ap()
    wrtab_in = nc.dram_tensor("wrtab", [N_RELATION, DIM], bf16,
                              kind="ExternalInput").ap()
    ident_in = nc.dram_tensor("ident_in", [P, P], f32, kind="ExternalInput").ap()
    b2_in = nc.dram_tensor("b2", [P, 1], f32, kind="ExternalInput").ap()
    sels_in = nc.dram_tensor("sels", [P, NCHUNK * NR], f32, kind="ExternalInput").ap()
    par2_in = nc.dram_tensor("par2", [P, 2], f32, kind="ExternalInput").ap()
    parT_in = nc.dram_tensor("parT", [2, P], f32, kind="ExternalInput").ap()
    out_dram = nc.dram_tensor("scores", [NR, 16], f32, kind="ExternalOutput").ap()

    with tile.TileContext(nc) as tc:
        with (
            tc.tile_pool(name="const", bufs=1) as cpool,
            tc.tile_pool(name="idx", bufs=2) as ipool,
            tc.tile_pool(name="gath", bufs=2) as gpool,
            tc.tile_pool(name="work", bufs=2) as wpool,
            tc.tile_pool(name="small", bufs=2) as spool,
            tc.tile_pool(name="psO", bufs=1, space="PSUM") as poolO,
            tc.tile_pool(name="psT", bufs=2, space="PSUM") as poolT,
            tc.tile_pool(name="psR", bufs=1, space="PSUM") as poolR,
            tc.tile_pool(name="psS", bufs=1, space="PSUM") as poolS,
        ):
            # final-gather indices FIRST in the sync queue: the Pool gather
            # stream opens with the final gathers, which wait only on these.
            fu = ipool.tile([P, 4], i32, tag="fu")
            nc.sync.dma_start(out=fu[:], in_=fin_users[:, :])
            fi = ipool.tile([P, 4], i32, tag="fi")
            nc.sync.dma_start(out=fi[:], in_=fin_items[:, :])

            # ---------------- constants to SBUF
            # (identity via HWDGE load — make_identity's gpsimd affine_select
            # would stall the Pool gather stream with an ext-isa IRAM load)
            ident = cpool.tile([P, P], f32, tag="ident")
            nc.sync.dma_start(out=ident[:], in_=ident_in[:, :])
            wh_t = cpool.tile([P, P], f32, tag="wh")
            nc.sync.dma_start(out=wh_t[:], in_=wh_bd[:, :])
            whh_t = cpool.tile([P, P], f32, tag="whh")
            nc.sync.dma_start(out=whh_t[:], in_=whh_bd[:, :])
            wrtab_t = cpool.tile([N_RELATION, DIM], bf16, tag="wrtab")
            nc.sync.dma_start(out=wrtab_t[:], in_=wrtab_in[:, :])
            b2_t = cpool.tile([P, 1], f32, tag="b2")
            nc.sync.dma_start(out=b2_t[:], in_=b2_in[:, :])
            sels_t = cpool.tile([P, NCHUNK * NR], f32, tag="sels")
            nc.sync.dma_start(out=sels_t[:], in_=sels_in[:, :])
            par2_t = cpool.tile([P, 2], f32, tag="par2")
            nc.sync.dma_start(out=par2_t[:], in_=par2_in[:, :])
            parT_t = cpool.tile([2, P], f32, tag="parT")
            nc.sync.dma_start(out=parT_t[:], in_=parT_in[:, :])

            # index + rrsq tiles, loaded up front in a few big DMAs
            # (hop indices first: the hop pipeline starts before hop0 now)
            ih_full = [cpool.tile([P, J], i32, tag=f"ihf{l}", name=f"ihf{l}")
                       for l in range(L)]
            it_full = [cpool.tile([P, J], i32, tag=f"itf{l}", name=f"itf{l}")
                       for l in range(L)]
            rr_full = [cpool.tile([P, J], f32, tag=f"rrf{l}", name=f"rrf{l}")
                       for l in range(L)]
            for l in range(L):
                nc.sync.dma_start(out=ih_full[l][:], in_=idx_h[l, :, :])
                nc.sync.dma_start(out=it_full[l][:], in_=idx_t[l, :, :])
                nc.sync.dma_start(out=rr_full[l][:], in_=rrsq_in[l, :, :])
            idx0_full = cpool.tile([P, J], i32, tag="idx0f")
            nc.sync.dma_start(out=idx0_full[:], in_=idx_hop0[:, :])

            # persistent output accumulator [NR, 512] (one PSUM bank)
            o_ps = poolO.tile([NR, 512], f32, tag="o")
            first_omm = [True]

            def o_accum(rhs_ap, chunk, is_last):
                """rhs [128, 512] -> accumulate selector chunk into o_ps."""
                nc.tensor.matmul(
                    out=o_ps[:, :],
                    lhsT=sels_t[:, chunk * NR:(chunk + 1) * NR],
                    rhs=rhs_ap,
                    start=first_omm[0],
                    stop=is_last,
                    skip_group_check=True,
                )
                first_omm[0] = False

            # ---------------- final gathers (independent of hops; issue early)
            # [128, 4] index layout -> full 128-descriptor instructions (8 total),
            # then partition-coalescing SBUF->SBUF DMAs restore the G-format:
            # gather position (p, cb) holds user at G row r=p//4, slot j=cb*4+p%4.
            ru_p = spool.tile([P, 4 * DIM], f32, tag="rup")
            ie_p = spool.tile([P, 4 * DIM], f32, tag="iep")
            for cb in range(4):
                sl = slice(cb * DIM, (cb + 1) * DIM)
                nc.gpsimd.indirect_dma_start(
                    out=ru_p[:, sl], out_offset=None, in_=rec_user[:, :],
                    in_offset=bass.IndirectOffsetOnAxis(
                        ap=fu[:, cb:cb + 1], axis=0))
                nc.gpsimd.indirect_dma_start(
                    out=ie_p[:, sl], out_offset=None, in_=item_comb[:, :],
                    in_offset=bass.IndirectOffsetOnAxis(
                        ap=fi[:, cb:cb + 1], axis=0))
            ru_g = spool.tile([NR, 512], f32, tag="ru")
            ie_g = spool.tile([NR, 512], f32, tag="ie")
            for cb in range(4):
                nc.sync.dma_start(
                    out=ru_g[:, cb * 128:(cb + 1) * 128],
                    in_=ru_p[:, cb * DIM:(cb + 1) * DIM])
                nc.sync.dma_start(
                    out=ie_g[:, cb * 128:(cb + 1) * 128],
                    in_=ie_p[:, cb * DIM:(cb + 1) * DIM])
            # ru.ie partial dot, computed early (hidden under the gather stream)
            prB = spool.tile([NR, 512], f32, tag="prB")
            nc.vector.tensor_tensor(out=prB[:], in0=ru_g[:], in1=ie_g[:],
                                    op=mybir.AluOpType.mult)
            dotB = spool.tile([NR, 16], f32, tag="dotB")
            nc.vector.tensor_reduce(
                out=dotB[:], in_=prB[:].rearrange("p (j d) -> p j d", d=DIM),
                axis=mybir.AxisListType.X, op=mybir.AluOpType.add)

            # ---------------- hops
            for l in range(L):
                for b in range(NBATCH):
                    jlo = b * JB
                    # one-hot(r) for this batch [64, JB*128] bf16
                    oh = gpool.tile([N_RELATION, JB * P], bf16, tag="oh")
                    nc.sync.dma_start(
                        out=oh[:], in_=onehot_in[l, :, jlo * P:(jlo + JB) * P])
                    # entity gathers: one indirect DMA (128 rows) per j-column
                    Hg = gpool.tile([P, JB * DIM], f32, tag="h")
                    Tg = gpool.tile([P, JB * DIM], f32, tag="t")
                    for jj in range(JB):
                        sl = slice(jj * DIM, (jj + 1) * DIM)
                        nc.gpsimd.indirect_dma_start(
                            out=Hg[:, sl], out_offset=None, in_=entity[:, :],
                            in_offset=bass.IndirectOffsetOnAxis(
                                ap=ih_full[l][:, jlo + jj:jlo + jj + 1], axis=0))
                        nc.gpsimd.indirect_dma_start(
                            out=Tg[:, sl], out_offset=None, in_=entity[:, :],
                            in_offset=bass.IndirectOffsetOnAxis(
                                ap=it_full[l][:, jlo + jj:jlo + jj + 1], axis=0))

                    # ---- logits: dht + rrsq ; pi = softmax_k
                    prod = wpool.tile([P, JB * DIM], f32, tag="prod")
                    nc.vector.tensor_tensor(
                        out=prod[:], in0=Hg[:], in1=Tg[:], op=mybir.AluOpType.mult)
                    dht = spool.tile([P, JB], f32, tag="dht")
                    nc.vector.tensor_reduce(
                        out=dht[:], in_=prod[:].rearrange("p (j d) -> p j d", d=DIM),
                        axis=mybir.AxisListType.X, op=mybir.AluOpType.add)
                    logits = spool.tile([P, JB], f32, tag="lg")
                    nc.vector.tensor_tensor(
                        out=logits[:], in0=dht[:],
                        in1=rr_full[l][:, jlo:jlo + JB], op=mybir.AluOpType.add)
                    E = spool.tile([P, JB], f32, tag="E")
                    nc.scalar.activation(
                        out=E[:], in_=logits[:], func=mybir.ActivationFunctionType.Exp)
                    # denominators: [2, JB] = parity sums of E
                    den_ps = poolS.tile([2, JB], f32, tag="dn")
                    nc.tensor.matmul(out=den_ps[:], lhsT=par2_t[:], rhs=E[:],
                                     start=True, stop=True)
                    rec = spool.tile([2, JB], f32, tag="rec")
                    nc.vector.reciprocal(out=rec[:], in_=den_ps[:])
                    rb_ps = poolS.tile([P, JB], f32, tag="rb")
                    nc.tensor.matmul(out=rb_ps[:], lhsT=parT_t[:], rhs=rec[:],
                                     start=True, stop=True)
                    pi = spool.tile([P, JB], f32, tag="pi")
                    nc.vector.tensor_tensor(
                        out=pi[:], in0=E[:], in1=rb_ps[:], op=mybir.AluOpType.mult)

                    # ---- transposes to feature-major (4 blocks per psT bank)
                    HgT = wpool.tile([P, JB * DIM], f32, tag="hT")
                    TgT = wpool.tile([P, JB * DIM], f32, tag="tT")
                    for (src, dst, ei) in ((Hg, HgT, 0), (Tg, TgT, 1)):
                        for g in range(STB // 4):  # bank groups
                            tp = poolT.tile([P, 512], f32, tag="tp")
                            for q in range(4):
                                st = g * 4 + q
                                nc.tensor.transpose(
                                    out=tp[:, q * 128:(q + 1) * 128],
                                    in_=src[:, st * 128:(st + 1) * 128],
                                    identity=ident[:])
                            if (g + ei) % 2 == 0:
                                nc.vector.tensor_copy(
                                    out=dst[:, g * 512:(g + 1) * 512], in_=tp[:])
                            else:
                                nc.scalar.copy(
                                    out=dst[:, g * 512:(g + 1) * 512], in_=tp[:])

                    # ---- RNN step 1: A = Wh*H^T + WRtab^T@onehot ; h1=tanh(A+b2)
                    A_ps = poolR.tile([P, JB * DIM], f32, tag="rnn")
                    for st in range(STB):
                        for q in range(4):
                            jc = st * 4 + q
                            nc.tensor.matmul(
                                out=A_ps[q * DIM:(q + 1) * DIM,
                                         st * 128:(st + 1) * 128],
                                lhsT=wrtab_t[:],
                                rhs=oh[:, jc * P:(jc + 1) * P],
                                start=True, stop=False,
                                tile_position=(0, q * DIM),
                                skip_group_check=True)
                    for st in range(STB):
                        nc.tensor.matmul(
                            out=A_ps[:, st * 128:(st + 1) * 128], lhsT=wh_t[:],
                            rhs=HgT[:, st * 128:(st + 1) * 128],
                            start=False, stop=(st % 4 == 3),
                            skip_group_check=True)
                    h1 = wpool.tile([P, JB * DIM], f32, tag="h1")
                    nc.scalar.activation(
                        out=h1[:], in_=A_ps[:],
                        func=mybir.ActivationFunctionType.Tanh, bias=b2_t[:, :])

                    # ---- RNN step 2: B = Wh*T^T + Whh*h1 + WRtab^T@onehot
                    B_ps = poolR.tile([P, JB * DIM], f32, tag="rnn")
                    for st in range(STB):
                        for q in range(4):
                            jc = st * 4 + q
                            nc.tensor.matmul(
                                out=B_ps[q * DIM:(q + 1) * DIM,
                                         st * 128:(st + 1) * 128],
                                lhsT=wrtab_t[:],
                                rhs=oh[:, jc * P:(jc + 1) * P],
                                start=True, stop=False,
                                tile_position=(0, q * DIM),
                                skip_group_check=True)
                    for st in range(STB):
                        nc.tensor.matmul(
                            out=B_ps[:, st * 128:(st + 1) * 128], lhsT=wh_t[:],
                            rhs=TgT[:, st * 128:(st + 1) * 128],
                            start=False, stop=False,
                            skip_group_check=True)
                    for st in range(STB):
                        nc.tensor.matmul(
                            out=B_ps[:, st * 128:(st + 1) * 128], lhsT=whh_t[:],
                            rhs=h1[:, st * 128:(st + 1) * 128],
                            start=False, stop=(st % 4 == 3),
                            skip_group_check=True)
                    h2T = wpool.tile([P, JB * DIM], f32, tag="h2T")
                    nc.scalar.activation(
                        out=h2T[:], in_=B_ps[:],
                        func=mybir.ActivationFunctionType.Tanh, bias=b2_t[:, :])

                    # ---- back to token-major, scale by pi, accumulate into o
                    C_ps = poolR.tile([P, JB * DIM], f32, tag="rnn")
                    for st in range(STB):
                        nc.tensor.transpose(
                            out=C_ps[:, st * 128:(st + 1) * 128],
                            in_=h2T[:, st * 128:(st + 1) * 128], identity=ident[:])
                    scaled = wpool.tile([P, JB * DIM], f32, tag="sc")
                    for c in range(CPB):
                        nc.vector.tensor_tensor(
                            out=scaled[:, c * 512:(c + 1) * 512].rearrange(
                                "p (j d) -> p j d", d=DIM),
                            in0=C_ps[:, c * 512:(c + 1) * 512].rearrange(
                                "p (j d) -> p j d", d=DIM),
                            in1=pi[:, c * 16:(c + 1) * 16][:, :, None].to_broadcast(
                                [P, 16, DIM]),
                            op=mybir.AluOpType.mult)
                    for c in range(CPB):
                        o_accum(scaled[:, c * 512:(c + 1) * 512], b * CPB + c,
                                False)

            # ---------------- hop 0 LAST: its short selector-sum tail hides
            # the final hop batch's RNN drain under hop0's gather stream.
            for b in range(NBATCH):
                g0 = gpool.tile([P, JB * DIM], f32, tag="h")
                for jj in range(JB):
                    nc.gpsimd.indirect_dma_start(
                        out=g0[:, jj * DIM:(jj + 1) * DIM],
                        out_offset=None,
                        in_=entity[:, :],
                        in_offset=bass.IndirectOffsetOnAxis(
                            ap=idx0_full[:, b * JB + jj:b * JB + jj + 1], axis=0),
                    )
                for c in range(CPB):
                    o_accum(g0[:, c * 512:(c + 1) * 512], b * CPB + c,
                            (b == NBATCH - 1) and (c == CPB - 1))

            # ---------------- final: sigmoid(o.ie + ru.ie) (ru.ie precomputed)
            pr = spool.tile([NR, 512], f32, tag="pr")
            nc.vector.tensor_tensor(out=pr[:], in0=o_ps[:], in1=ie_g[:],
                                    op=mybir.AluOpType.mult)
            sc = spool.tile([NR, 16], f32, tag="scs")
            nc.vector.tensor_reduce(
                out=sc[:], in_=pr[:].rearrange("p (j d) -> p j d", d=DIM),
                axis=mybir.AxisListType.X, op=mybir.AluOpType.add)
            sc2 = spool.tile([NR, 16], f32, tag="sc2")
            nc.vector.tensor_tensor(out=sc2[:], in0=sc[:], in1=dotB[:],
                                    op=mybir.AluOpType.add)
            sg = spool.tile([NR, 16], f32, tag="sg")
            nc.scalar.activation(out=sg[:], in_=sc2[:],
                                 func=mybir.ActivationFunctionType.Sigmoid)
            nc.sync.dma_start(out=out_dram[:, :], in_=sg[:])

    nc.compile()
    return nc


# ---------------------------------------------------------------- host prep
def _prep_core_inputs(c, BC, users, items, hop0_items, heads, relations, tails,
                      entity_emb, relation_emb, rec_user_emb, rec_item_emb,
                      W_ih, W_hh, b_ih, b_hh, JB=32):
    """numpy preprocessing: shard + index-layout permutations + const matrices."""
    import ml_dtypes
    J = BC // 2
    NCHUNK = J // 16
    NR = 2 * NCHUNK
    lo, hi = c * BC, (c + 1) * BC

    def glayout(a, dtype=np.int32):  # [BC, K] -> [128, J]
        return np.ascontiguousarray(
            a.reshape(J, 2, K).transpose(1, 2, 0).reshape(P, J)).astype(dtype)

    def flayout2(a):  # [BC] -> [128, 4]: (p, cb) holds u at G (r=p//4, j=cb*4+p%4)
        p = np.arange(P)[:, None]
        cb = np.arange(4)[None, :]
        r, q = p // 4, p % 4
        j = cb * 4 + q
        u = (r // 2) * 32 + j * 2 + (r % 2)
        return np.ascontiguousarray(np.asarray(a)[u]).astype(np.int32)

    idx_h = np.stack([glayout(heads[l, lo:hi]) for l in range(L)])
    idx_t = np.stack([glayout(tails[l, lo:hi]) for l in range(L)])
    rel_g = np.stack([glayout(relations[l, lo:hi]) for l in range(L)])  # [L,128,J]

    # rrsq per token (host lookup of the 64-entry r.r table — index transform)
    rr_tab = (relation_emb.astype(np.float64) ** 2).sum(axis=1).astype(np.float32)
    rrsq = rr_tab[rel_g]                      # [L, 128, J] f32

    # onehot(r) in feature-major column order: [L, 64, J*128] bf16,
    # column j*128 + p <-> token (p, j)
    oh = (rel_g[:, None, :, :] == np.arange(N_RELATION)[None, :, None, None])
    oh = oh.transpose(0, 1, 3, 2).reshape(L, N_RELATION, J * P)
    oh = np.ascontiguousarray(oh).astype(ml_dtypes.bfloat16)

    Wh = W_ih[:, :DIM]
    Wr = W_ih[:, DIM:]
    # WRtab[rel] = Wr @ relation_emb[rel]  (so lhsT=WRtab gives WRtab^T @ onehot)
    wrtab = (relation_emb @ Wr.T).astype(ml_dtypes.bfloat16)   # [64, 32]

    def blockdiag(w):  # w: [32, 32] block = w.T
        m = np.zeros((P, P), np.float32)
        for j in range(4):
            m[j * 32:(j + 1) * 32, j * 32:(j + 1) * 32] = w.T
        return m

    b2 = np.tile((b_ih + b_hh).astype(np.float32), 4)[:, None]

    sels = np.zeros((P, NCHUNK, NR), np.float32)
    pvec = np.arange(P) // 64  # parity of each partition
    for m in range(NCHUNK):
        for p in range(P):
            sels[p, m, 2 * m + pvec[p]] = 1.0
    par2 = np.zeros((P, 2), np.float32)
    par2[np.arange(P), pvec] = 1.0

    return {
        "entity": np.ascontiguousarray(entity_emb, np.float32),
        "rec_user": np.ascontiguousarray(rec_user_emb, np.float32),
        "item_comb": np.ascontiguousarray(
            entity_emb[:N_ITEM] + rec_item_emb, np.float32),
        "idx_hop0": glayout(hop0_items[lo:hi]),
        "idx_h": idx_h, "idx_t": idx_t,
        "rrsq": rrsq,
        "onehot": oh,
        "fin_users": flayout2(users[lo:hi]),
        "fin_items": flayout2(items[lo:hi]),
        "wh_bd": blockdiag(Wh),
        "whh_bd": blockdiag(W_hh),
        "wrtab": wrtab,
        "ident_in": np.eye(P, dtype=np.float32),
        "b2": b2,
        "sels": np.ascontiguousarray(sels.reshape(P, NCHUNK * NR)),
        "par2": par2,
        "parT": np.ascontiguousarray(par2.T),
    }


def _unscramble(out_c, BC):
    """[NR, 16] core output -> [BC] user scores."""
    NCHUNK = (BC // 2) // 16
    return np.ascontiguousarray(
        out_c.reshape(NCHUNK, 2, 16).transpose(0, 2, 1).reshape(BC))


_CACHED = {}
TRACE = False  # set True (e.g. from test.py) to capture an NTFF profile
LAST_RESULTS = None


def kernel(**inputs):
    global LAST_RESULTS
    from concourse import bass_utils

    BC = B // NCORES
    if "nc" not in _CACHED:
        _CACHED["nc"] = build_core_program(BC=BC)
    nc = _CACHED["nc"]

    args = {k: np.asarray(v) for k, v in inputs.items()}
    in_maps = [
        _prep_core_inputs(
            c, BC,
            args["users"], args["items"], args["hop0_items"], args["heads"],
            args["relations"], args["tails"],
            np.asarray(args["entity_emb"], np.float32),
            np.asarray(args["relation_emb"], np.float32),
            np.asarray(args["rec_user_emb"], np.float32),
            np.asarray(args["rec_item_emb"], np.float32),
            np.asarray(args["W_ih"], np.float32),
            np.asarray(args["W_hh"], np.float32),
            np.asarray(args["b_ih"], np.float32),
            np.asarray(args["b_hh"], np.float32),
        )
        for c in range(NCORES)
    ]
    res = bass_utils.run_bass_kernel_spmd(
        nc, in_maps, core_ids=list(range(NCORES)), trace=TRACE)
    LAST_RESULTS = res
    out = np.concatenate(
        [_unscramble(res.results[c]["scores"], BC) for c in range(NCORES)])
    return out



# revision 4
# speedup vs baseline: 1.0005x; 1.0005x over previous
"""Trainium2 Bass kernel for nn_NCFG_21139829031662 (gnn_message_passing).

RippleNet-style model: hop-0 seed-set sum + 2 hops of (gather triples,
attention softmax over K, 2-step tanh RNN, weighted sum), then a
user/item dot + sigmoid.

Strategy: pure data-parallel over the 4096-user batch across 8 cores
(512 users/core); embedding tables replicated in each core's HBM.
The dominant cost is SWDGE descriptor generation for the 128B random
entity gathers (~10.8 ns/row on the GpSimd Q7), so the kernel
eliminates every non-entity SWDGE instruction:
  - relation embeddings never touch SWDGE: r.r enters the logits as a
    host-precomputed per-token scalar (an index transform), and the RNN's
    Wr@r term is computed on the PE as WRtab^T @ onehot(r) accumulated
    straight into the RNN PSUM groups (onehot uploaded as bf16 0/1).
  - entity/hop0/final gathers use per-column indirect1d DMAs (the HW
    contract: 128 descriptors per instruction).
  - the final item embedding entity_emb[items] + rec_item_emb[items] is
    gathered once from a host-precombined table (same-index fold).

Per-core on-chip layout ("G-layout"): token (u, k) -> partition
p = (u%2)*64 + k, free column j = u//2 (32 f32 per column).
"""

import sys
import numpy as np

sys.path.insert(0, "/opt/trn_rl_repo")

# ---------------------------------------------------------------- constants
DIM = 32
N_ENTITY = 500000
N_RELATION = 64
N_USER = 100000
N_ITEM = 200000
B = 4096
K = 64
L = 2
NCORES = 8
P = 128


def build_core_program(BC=512, JB=32):
    """Build the single-core bass program (SPMD: same program on all cores).

    BC: users per core. JB: j-columns (user pairs) per processing batch.
    """
    import concourse.bass as bass
    import concourse.bacc as bacc
    import concourse.mybir as mybir
    import concourse.tile as tile
    from concourse.masks import make_identity

    J = BC // 2              # j-columns total
    NBATCH = J // JB         # batches per hop
    NCHUNK = J // 16         # 16-j output chunks
    NR = 2 * NCHUNK          # output psum rows
    assert J % JB == 0 and JB % 16 == 0
    CPB = JB // 16           # chunks per batch
    STB = JB // 4            # supertiles ([128,128] blocks) per batch
    f32 = mybir.dt.float32
    bf16 = mybir.dt.bfloat16
    i32 = mybir.dt.int32

    nc = bacc.Bacc("TRN2", target_bir_lowering=False, debug=False)

    # DRAM inputs
    entity = nc.dram_tensor("entity", [N_ENTITY, DIM], f32, kind="ExternalInput").ap()
    rec_user = nc.dram_tensor("rec_user", [N_USER, DIM], f32, kind="ExternalInput").ap()
    # host-precomputed entity_emb[:N_ITEM] + rec_item_emb (same-index fold)
    item_comb = nc.dram_tensor("item_comb", [N_ITEM, DIM], f32,
                               kind="ExternalInput").ap()
    idx_hop0 = nc.dram_tensor("idx_hop0", [P, J], i32, kind="ExternalInput").ap()
    idx_h = nc.dram_tensor("idx_h", [L, P, J], i32, kind="ExternalInput").ap()
    idx_t = nc.dram_tensor("idx_t", [L, P, J], i32, kind="ExternalInput").ap()
    rrsq_in = nc.dram_tensor("rrsq", [L, P, J], f32, kind="ExternalInput").ap()
    onehot_in = nc.dram_tensor("onehot", [L, N_RELATION, J * P], bf16,
                               kind="ExternalInput").ap()
    fin_users = nc.dram_tensor("fin_users", [P, 4], i32, kind="ExternalInput").ap()
    fin_items = nc.dram_tensor("fin_items", [P, 4], i32, kind="ExternalInput").ap()
    wh_bd = nc.dram_tensor("wh_bd", [P, P], f32, kind="ExternalInput").ap()
    whh_bd = nc.dram_tensor("whh_bd", [P, P], f32, kind="ExternalInput").ap()
    wrtab_in = nc.dram_tensor("wrtab", [N_RELATION, DIM], bf16,
                              kind="ExternalInput").ap()
    ident_in = nc.dram_tensor("ident_in", [P, P], f32, kind="ExternalInput").ap()
    b2_in = nc.dram_tensor("b2", [P, 1], f32, kind="ExternalInput").ap()
    sels_in = nc.dram_tensor("sels", [P, NCHUNK * NR], f32, kind="ExternalInput").ap()
    par2_in = nc.dram_tensor("par2", [P, 2], f32, kind="ExternalInput").ap()
    parT_in = nc.dram_tensor("parT", [2, P], f32, kind="ExternalInput").ap()
    out_dram = nc.dram_tensor("scores", [NR, 16], f32, kind="ExternalOutput").ap()

    with tile.TileContext(nc) as tc:
        with (
            tc.tile_pool(name="const", bufs=1) as cpool,
            tc.tile_pool(name="idx", bufs=2) as ipool,
            tc.tile_pool(name="gath", bufs=2) as gpool,
            tc.tile_pool(name="work", bufs=2) as wpool,
            tc.tile_pool(name="small", bufs=2) as spool,
            tc.tile_pool(name="psO", bufs=1, space="PSUM") as poolO,
            tc.tile_pool(name="psT", bufs=2, space="PSUM") as poolT,
            tc.tile_pool(name="psR", bufs=1, space="PSUM") as poolR,
            tc.tile_pool(name="psS", bufs=1, space="PSUM") as poolS,
        ):
            # final-gather indices FIRST in the sync queue: the Pool gather
            # stream opens with the final gathers, which wait only on these.
            fu = ipool.tile([P, 4], i32, tag="fu")
            nc.sync.dma_start(out=fu[:], in_=fin_users[:, :])
            fi = ipool.tile([P, 4], i32, tag="fi")
            nc.sync.dma_start(out=fi[:], in_=fin_items[:, :])

            # ---------------- constants to SBUF
            # (identity via HWDGE load — make_identity's gpsimd affine_select
            # would stall the Pool gather stream with an ext-isa IRAM load)
            ident = cpool.tile([P, P], f32, tag="ident")
            nc.sync.dma_start(out=ident[:], in_=ident_in[:, :])
            wh_t = cpool.tile([P, P], f32, tag="wh")
            nc.sync.dma_start(out=wh_t[:], in_=wh_bd[:, :])
            whh_t = cpool.tile([P, P], f32, tag="whh")
            nc.sync.dma_start(out=whh_t[:], in_=whh_bd[:, :])
            wrtab_t = cpool.tile([N_RELATION, DIM], bf16, tag="wrtab")
            nc.sync.dma_start(out=wrtab_t[:], in_=wrtab_in[:, :])
            b2_t = cpool.tile([P, 1], f32, tag="b2")
            nc.sync.dma_start(out=b2_t[:], in_=b2_in[:, :])
            sels_t = cpool.tile([P, NCHUNK * NR], f32, tag="sels")
            nc.sync.dma_start(out=sels_t[:], in_=sels_in[:, :])
            par2_t = cpool.tile([P, 2], f32, tag="par2")
            nc.sync.dma_start(out=par2_t[:], in_=par2_in[:, :])
            parT_t = cpool.tile([2, P], f32, tag="parT")
            nc.sync.dma_start(out=parT_t[:], in_=parT_in[:, :])

            # index + rrsq tiles, loaded up front in a few big DMAs
            # (hop indices first: the hop pipeline starts before hop0 now)
            ih_full = [cpool.tile([P, J], i32, tag=f"ihf{l}", name=f"ihf{l}")
                       for l in range(L)]
            it_full = [cpool.tile([P, J], i32, tag=f"itf{l}", name=f"itf{l}")
                       for l in range(L)]
            rr_full = [cpool.tile([P, J], f32, tag=f"rrf{l}", name=f"rrf{l}")
                       for l in range(L)]
            for l in range(L):
                nc.sync.dma_start(out=ih_full[l][:], in_=idx_h[l, :, :])
                nc.sync.dma_start(out=it_full[l][:], in_=idx_t[l, :, :])
                nc.sync.dma_start(out=rr_full[l][:], in_=rrsq_in[l, :, :])
            idx0_full = cpool.tile([P, J], i32, tag="idx0f")
            nc.sync.dma_start(out=idx0_full[:], in_=idx_hop0[:, :])

            # persistent output accumulator [NR, 512] (one PSUM bank)
            o_ps = poolO.tile([NR, 512], f32, tag="o")
            first_omm = [True]

            def o_accum(rhs_ap, chunk, is_last):
                """rhs [128, 512] -> accumulate selector chunk into o_ps."""
                nc.tensor.matmul(
                    out=o_ps[:, :],
                    lhsT=sels_t[:, chunk * NR:(chunk + 1) * NR],
                    rhs=rhs_ap,
                    start=first_omm[0],
                    stop=is_last,
                    skip_group_check=True,
                )
                first_omm[0] = False

            # ---------------- final gathers (independent of hops; issue early)
            # [128, 4] index layout -> full 128-descriptor instructions (8 total),
            # then partition-coalescing SBUF->SBUF DMAs restore the G-format:
            # gather position (p, cb) holds user at G row r=p//4, slot j=cb*4+p%4.
            ru_p = spool.tile([P, 4 * DIM], f32, tag="rup")
            ie_p = spool.tile([P, 4 * DIM], f32, tag="iep")
            for cb in range(4):
                sl = slice(cb * DIM, (cb + 1) * DIM)
                nc.gpsimd.indirect_dma_start(
                    out=ru_p[:, sl], out_offset=None, in_=rec_user[:, :],
                    in_offset=bass.IndirectOffsetOnAxis(
                        ap=fu[:, cb:cb + 1], axis=0))
                nc.gpsimd.indirect_dma_start(
                    out=ie_p[:, sl], out_offset=None, in_=item_comb[:, :],
                    in_offset=bass.IndirectOffsetOnAxis(
                        ap=fi[:, cb:cb + 1], axis=0))
            ru_g = spool.tile([NR, 512], f32, tag="ru")
            ie_g = spool.tile([NR, 512], f32, tag="ie")
            for cb in range(4):
                nc.sync.dma_start(
                    out=ru_g[:, cb * 128:(cb + 1) * 128],
                    in_=ru_p[:, cb * DIM:(cb + 1) * DIM])
                nc.sync.dma_start(
                    out=ie_g[:, cb * 128:(cb + 1) * 128],
                    in_=ie_p[:, cb * DIM:(cb + 1) * DIM])
            # ru.ie partial dot, computed early (hidden under the gather stream)
            prB = spool.tile([NR, 512], f32, tag="prB")
            nc.vector.tensor_tensor(out=prB[:], in0=ru_g[:], in1=ie_g[:],
                                    op=mybir.AluOpType.mult)
            dotB = spool.tile([NR, 16], f32, tag="dotB")
            nc.vector.tensor_reduce(
                out=dotB[:], in_=prB[:].rearrange("p (j d) -> p j d", d=DIM),
                axis=mybir.AxisListType.X, op=mybir.AluOpType.add)

            # ---------------- hops
            for l in range(L):
                for b in range(NBATCH):
                    jlo = b * JB
                    # one-hot(r) for this batch [64, JB*128] bf16
                    oh = gpool.tile([N_RELATION, JB * P], bf16, tag="oh")
                    nc.sync.dma_start(
                        out=oh[:], in_=onehot_in[l, :, jlo * P:(jlo + JB) * P])
                    # entity gathers: one indirect DMA (128 rows) per j-column
                    Hg = gpool.tile([P, JB * DIM], f32, tag="h")
                    Tg = gpool.tile([P, JB * DIM], f32, tag="t")
                    for jj in range(JB):
                        sl = slice(jj * DIM, (jj + 1) * DIM)
                        nc.gpsimd.indirect_dma_start(
                            out=Hg[:, sl], out_offset=None, in_=entity[:, :],
                            in_offset=bass.IndirectOffsetOnAxis(
                                ap=ih_full[l][:, jlo + jj:jlo + jj + 1], axis=0))
                        nc.gpsimd.indirect_dma_start(
                            out=Tg[:, sl], out_offset=None, in_=entity[:, :],
                            in_offset=bass.IndirectOffsetOnAxis(
                                ap=it_full[l][:, jlo + jj:jlo + jj + 1], axis=0))

                    # ---- logits: dht + rrsq ; pi = softmax_k
                    prod = wpool.tile([P, JB * DIM], f32, tag="prod")
                    nc.vector.tensor_tensor(
                        out=prod[:], in0=Hg[:], in1=Tg[:], op=mybir.AluOpType.mult)
                    dht = spool.tile([P, JB], f32, tag="dht")
                    nc.vector.tensor_reduce(
                        out=dht[:], in_=prod[:].rearrange("p (j d) -> p j d", d=DIM),
                        axis=mybir.AxisListType.X, op=mybir.AluOpType.add)
                    logits = spool.tile([P, JB], f32, tag="lg")
                    nc.vector.tensor_tensor(
                        out=logits[:], in0=dht[:],
                        in1=rr_full[l][:, jlo:jlo + JB], op=mybir.AluOpType.add)
                    E = spool.tile([P, JB], f32, tag="E")
                    nc.scalar.activation(
                        out=E[:], in_=logits[:], func=mybir.ActivationFunctionType.Exp)
                    # denominators: [2, JB] = parity sums of E
                    den_ps = poolS.tile([2, JB], f32, tag="dn")
                    nc.tensor.matmul(out=den_ps[:], lhsT=par2_t[:], rhs=E[:],
                                     start=True, stop=True)
                    rec = spool.tile([2, JB], f32, tag="rec")
                    nc.vector.reciprocal(out=rec[:], in_=den_ps[:])
                    rb_ps = poolS.tile([P, JB], f32, tag="rb")
                    nc.tensor.matmul(out=rb_ps[:], lhsT=parT_t[:], rhs=rec[:],
                                     start=True, stop=True)
                    pi = spool.tile([P, JB], f32, tag="pi")
                    nc.vector.tensor_tensor(
                        out=pi[:], in0=E[:], in1=rb_ps[:], op=mybir.AluOpType.mult)

                    # ---- transposes to feature-major (4 blocks per psT bank)
                    HgT = wpool.tile([P, JB * DIM], f32, tag="hT")
                    TgT = wpool.tile([P, JB * DIM], f32, tag="tT")
                    for (src, dst, ei) in ((Hg, HgT, 0), (Tg, TgT, 1)):
                        for g in range(STB // 4):  # bank groups
                            tp = poolT.tile([P, 512], f32, tag="tp")
                            for q in range(4):
                                st = g * 4 + q
                                nc.tensor.transpose(
                                    out=tp[:, q * 128:(q + 1) * 128],
                                    in_=src[:, st * 128:(st + 1) * 128],
                                    identity=ident[:])
                            if (g + ei) % 2 == 0:
                                nc.vector.tensor_copy(
                                    out=dst[:, g * 512:(g + 1) * 512], in_=tp[:])
                            else:
                                nc.scalar.copy(
                                    out=dst[:, g * 512:(g + 1) * 512], in_=tp[:])

                    # ---- RNN step 1: A = Wh*H^T + WRtab^T@onehot ; h1=tanh(A+b2)
                    A_ps = poolR.tile([P, JB * DIM], f32, tag="rnn")
                    for st in range(STB):
                        for q in range(4):
                            jc = st * 4 + q
                            nc.tensor.matmul(
                                out=A_ps[q * DIM:(q + 1) * DIM,
                                         st * 128:(st + 1) * 128],
                                lhsT=wrtab_t[:],
                                rhs=oh[:, jc * P:(jc + 1) * P],
                                start=True, stop=False,
                                tile_position=(0, q * DIM),
                                skip_group_check=True)
                    for st in range(STB):
                        nc.tensor.matmul(
                            out=A_ps[:, st * 128:(st + 1) * 128], lhsT=wh_t[:],
                            rhs=HgT[:, st * 128:(st + 1) * 128],
                            start=False, stop=(st % 4 == 3),
                            skip_group_check=True)
                    h1 = wpool.tile([P, JB * DIM], f32, tag="h1")
                    nc.scalar.activation(
                        out=h1[:], in_=A_ps[:],
                        func=mybir.ActivationFunctionType.Tanh, bias=b2_t[:, :])

                    # ---- RNN step 2: B = Wh*T^T + Whh*h1 + WRtab^T@onehot
                    B_ps = poolR.tile([P, JB * DIM], f32, tag="rnn")
                    for st in range(STB):
                        for q in range(4):
                            jc = st * 4 + q
                            nc.tensor.matmul(
                                out=B_ps[q * DIM:(q + 1) * DIM,
                                         st * 128:(st + 1) * 128],
                                lhsT=wrtab_t[:],
                                rhs=oh[:, jc * P:(jc + 1) * P],
                                start=True, stop=False,
                                tile_position=(0, q * DIM),
                                skip_group_check=True)
                    for st in range(STB):
                        nc.tensor.matmul(
                            out=B_ps[:, st * 128:(st + 1) * 128], lhsT=wh_t[:],
                            rhs=TgT[:, st * 128:(st + 1) * 128],
                            start=False, stop=False,
                            skip_group_check=True)
                    for st in range(STB):
                        nc.tensor.matmul(
                            out=B_ps[:, st * 128:(st + 1) * 128], lhsT=whh_t[:],
                            rhs=h1[:, st * 128:(st + 1) * 128],
                            start=False, stop=(st % 4 == 3),
                            skip_group_check=True)
                    h2T = wpool.tile([P, JB * DIM], f32, tag="h2T")
                    nc.scalar.activation(
                        out=h2T[:], in_=B_ps[:],
                        func=mybir.ActivationFunctionType.Tanh, bias=b2_t[:, :])

                    # ---- back to token-major, scale by pi, accumulate into o
                    C_ps = poolR.tile([P, JB * DIM], f32, tag="rnn")
                    for st in range(STB):
                        nc.tensor.transpose(
                            out=C_ps[:, st * 128:(st + 1) * 128],
                            in_=h2T[:, st * 128:(st + 1) * 128], identity=ident[:])
                    scaled = wpool.tile([P, JB * DIM], f32, tag="sc")
                    for c in range(CPB):
                        nc.vector.tensor_tensor(
                            out=scaled[:, c * 512:(c + 1) * 512].rearrange(
                                "p (j d) -> p j d", d=DIM),
                            in0=C_ps[:, c * 512:(c + 1) * 512].rearrange(
                                "p (j d) -> p j d", d=DIM),
                            in1=pi[:, c * 16:(c + 1) * 16][:, :, None].to_broadcast(
                                [P, 16, DIM]),
                            op=mybir.AluOpType.mult)
                    for c in range(CPB):
                        o_accum(scaled[:, c * 512:(c + 1) * 512], b * CPB + c,
                                False)

            # ---------------- hop 0 LAST: its short selector-sum tail hides
            # the final hop batch's RNN drain under hop0's gather stream.
            for b in range(NBATCH):
                g0 = gpool.tile([P, JB * DIM], f32, tag="h")
                for jj in range(JB):
                    nc.gpsimd.indirect_dma_start(
                        out=g0[:, jj * DIM:(jj + 1) * DIM],
                        out_offset=None,
                        in_=entity[:, :],
                        in_offset=bass.IndirectOffsetOnAxis(
                            ap=idx0_full[:, b * JB + jj:b * JB + jj + 1], axis=0),
                    )
                for c in range(CPB):
                    o_accum(g0[:, c * 512:(c + 1) * 512], b * CPB + c,
                            (b == NBATCH - 1) and (c == CPB - 1))

            # ---------------- final: sigmoid(o.ie + ru.ie) (ru.ie precomputed)
            pr = spool.tile([NR, 512], f32, tag="pr")
            nc.vector.tensor_tensor(out=pr[:], in0=o_ps[:], in1=ie_g[:],
                                    op=mybir.AluOpType.mult)
            sc = spool.tile([NR, 16], f32, tag="scs")
            nc.vector.tensor_reduce(
                out=sc[:], in_=pr[:].rearrange("p (j d) -> p j d", d=DIM),
                axis=mybir.AxisListType.X, op=mybir.AluOpType.add)
            sc2 = spool.tile([NR, 16], f32, tag="sc2")
            nc.vector.tensor_tensor(out=sc2[:], in0=sc[:], in1=dotB[:],
                                    op=mybir.AluOpType.add)
            sg = spool.tile([NR, 16], f32, tag="sg")
            nc.scalar.activation(out=sg[:], in_=sc2[:],
                                 func=mybir.ActivationFunctionType.Sigmoid)
            nc.sync.dma_start(out=out_dram[:, :], in_=sg[:])

    nc.compile()
    return nc


# ---------------------------------------------------------------- host prep
def _prep_core_inputs(c, BC, users, items, hop0_items, heads, relations, tails,
                      entity_emb, relation_emb, rec_user_emb, rec_item_emb,
                      W_ih, W_hh, b_ih, b_hh, JB=32):
    """numpy preprocessing: shard + index-layout permutations + const matrices."""
    import ml_dtypes
    J = BC // 2
    NCHUNK = J // 16
    NR = 2 * NCHUNK
    lo, hi = c * BC, (c + 1) * BC

    def glayout(a, dtype=np.int32):  # [BC, K] -> [128, J]
        return np.ascontiguousarray(
            a.reshape(J, 2, K).transpose(1, 2, 0).reshape(P, J)).astype(dtype)

    def flayout2(a):  # [BC] -> [128, 4]: (p, cb) holds u at G (r=p//4, j=cb*4+p%4)
        p = np.arange(P)[:, None]
        cb = np.arange(4)[None, :]
        r, q = p // 4, p % 4
        j = cb * 4 + q
        u = (r // 2) * 32 + j * 2 + (r % 2)
        return np.ascontiguousarray(np.asarray(a)[u]).astype(np.int32)

    idx_h = np.stack([glayout(heads[l, lo:hi]) for l in range(L)])
    idx_t = np.stack([glayout(tails[l, lo:hi]) for l in range(L)])
    rel_g = np.stack([glayout(relations[l, lo:hi]) for l in range(L)])  # [L,128,J]

    # rrsq per token (host lookup of the 64-entry r.r table — index transform)
    rr_tab = (relation_emb.astype(np.float64) ** 2).sum(axis=1).astype(np.float32)
    rrsq = rr_tab[rel_g]                      # [L, 128, J] f32

    # onehot(r) in feature-major column order: [L, 64, J*128] bf16,
    # column j*128 + p <-> token (p, j)
    oh = (rel_g[:, None, :, :] == np.arange(N_RELATION)[None, :, None, None])
    oh = oh.transpose(0, 1, 3, 2).reshape(L, N_RELATION, J * P)
    oh = np.ascontiguousarray(oh).astype(ml_dtypes.bfloat16)

    Wh = W_ih[:, :DIM]
    Wr = W_ih[:, DIM:]
    # WRtab[rel] = Wr @ relation_emb[rel]  (so lhsT=WRtab gives WRtab^T @ onehot)
    wrtab = (relation_emb @ Wr.T).astype(ml_dtypes.bfloat16)   # [64, 32]

    def blockdiag(w):  # w: [32, 32] block = w.T
        m = np.zeros((P, P), np.float32)
        for j in range(4):
            m[j * 32:(j + 1) * 32, j * 32:(j + 1) * 32] = w.T
        return m

    b2 = np.tile((b_ih + b_hh).astype(np.float32), 4)[:, None]

    sels = np.zeros((P, NCHUNK, NR), np.float32)
    pvec = np.arange(P) // 64  # parity of each partition
    for m in range(NCHUNK):
        for p in range(P):
            sels[p, m, 2 * m + pvec[p]] = 1.0
    par2 = np.zeros((P, 2), np.float32)
    par2[np.arange(P), pvec] = 1.0

    return {
        "entity": np.ascontiguousarray(entity_emb, np.float32),
        "rec_user": np.ascontiguousarray(rec_user_emb, np.float32),
        "item_comb": np.ascontiguousarray(
            entity_emb[:N_ITEM] + rec_item_emb, np.float32),
        "idx_hop0": glayout(hop0_items[lo:hi]),
        "idx_h": idx_h, "idx_t": idx_t,
        "rrsq": rrsq,
        "onehot": oh,
        "fin_users": flayout2(users[lo:hi]),
        "fin_items": flayout2(items[lo:hi]),
        "wh_bd": blockdiag(Wh),
        "whh_bd": blockdiag(W_hh),
        "wrtab": wrtab,
        "ident_in": np.eye(P, dtype=np.float32),
        "b2": b2,
        "sels": np.ascontiguousarray(sels.reshape(P, NCHUNK * NR)),
        "par2": par2,
        "parT": np.ascontiguousarray(par2.T),
    }


def _unscramble(out_c, BC):
    """[NR, 16] core output -> [BC] user scores."""
    NCHUNK = (BC // 2) // 16
    return np.ascontiguousarray(
        out_c.reshape(NCHUNK, 2, 16).transpose(0, 2, 1).reshape(BC))


_CACHED = {}
TRACE = False  # set True (e.g. from test.py) to capture an NTFF profile
LAST_RESULTS = None


def kernel(**inputs):
    global LAST_RESULTS
    from concourse import bass_utils

    BC = B // NCORES
    if "nc" not in _CACHED:
        _CACHED["nc"] = build_core_program(BC=BC)
    nc = _CACHED["nc"]

    args = {k: np.asarray(v) for k, v in inputs.items()}
    in_maps = [
        _prep_core_inputs(
            c, BC,
            args["users"], args["items"], args["hop0_items"], args["heads"],
            args["relations"], args["tails"],
            np.asarray(args["entity_emb"], np.float32),
            np.asarray(args["relation_emb"], np.float32),
            np.asarray(args["rec_user_emb"], np.float32),
            np.asarray(args["rec_item_emb"], np.float32),
            np.asarray(args["W_ih"], np.float32),
            np.asarray(args["W_hh"], np.float32),
            np.asarray(args["b_ih"], np.float32),
            np.asarray(args["b_hh"], np.float32),
        )
        for c in range(NCORES)
    ]
    res = bass_utils.run_bass_kernel_spmd(
        nc, in_maps, core_ids=list(range(NCORES)), trace=TRACE)
    LAST_RESULTS = res
    out = np.concatenate(
        [_unscramble(res.results[c]["scores"], BC) for c in range(NCORES)])
    return out

